# revision 54
# baseline (speedup 1.0000x reference)
"""Causal multi-head attention on 8 TRN2 NeuronCores — v2.

Problem: q,k,v [4, 16, 2048, 64] f32 -> out [4, 16, 2048, 64] f32
  out = softmax(causal(Q K^T / sqrt(64))) V  per (batch, head)

Sharding: 64 (b,h) pairs split across 8 cores (8 pairs per core), no
cross-core communication.

Host-side layout prep (inside kernel(), part of shard/layout staging):
  qT, kT: [BH, 64, S] fp16 (pre-transposed so d is the partition dim on
  device — no PE transposes needed), v65: [BH, S, 65] fp16 with a fused
  ones column (row 64 of O^T accumulates the softmax denominator).
  Output is written fp16 and upcast to f32 on the host.

Per-core device algorithm per (b,h) pair (S=2048, D=64, P=128, CW=512):
  - DMA qT/kT as [64, S] tiles and v as [128, NT, 65]; all fp16.
  - i-chunks of 4 query tiles (CW=512).  For each chunk, loop key tiles
    jb: score block ST[j, i] = K^T.T Q^T for the causal i-range, packed
    into [128, 1024] PSUM tiles (bank-aligned sub-slots), exp'd on the
    Scalar engine (scale fused) or DVE (Schraudolph fp16 bit-trick) into
    SBUF fp16.  Diagonal blocks get a triangular keep-mask on gpsimd.
  - PV uses E as the *stationary* operand: per (i-tile, jb),
    matmul(O_t[128, 65], lhsT=E[:, tile], rhs=V[jb]) accumulates in a
    per-i-tile PSUM bank.  Moving width is 65, not 128-512 — the cost
    model charges only moving columns, so this more than halves PV time
    vs the O^T formulation, and O lands directly in [i, c] layout (no
    epilogue transposes).
  - Chunk epilogue: rcp = 1/O[:, :, 64] and one broadcast multiply on
    DVE, fp16 out, DMA to DRAM.

Optimization landscape (measured via TimelineSim; defaults = 113423ns):
  - Engine budgets: ACT 100.9us busy (the pacer, 89%), PE 87.1us (hard
    fp16 floor: QK 17408 + PV 136x65 moving cols/head, 0.4167ns/col),
    DVE 79.8, Pool 47.7 (trimask only).
  - HARD CONSTRAINT: the BIR verifier rejects any GPSIMD (Pool) access
    to PSUM.  Scores and O-accumulators live in PSUM, so exp and the
    epilogue can only run on ACT/DVE.  (TimelineSim does NOT model this
    — a Pool-exp config sims at 103.5us but fails neuronxcc.)  Combined
    legal ACT+DVE work (~176us) / 2 ~= 88us/engine ~= the PE floor: a
    balanced split measures 115-124us (three coupled rho~1 pipelines
    with ST bufs 3 / po 2 of decoupling collapse), while the dominant-
    ACT baseline pays only ~12.5us of slop.
  - fp8 (DoubleRow, 0.5 cyc/col) on PV or QK is mechanically viable
    (paired-j-block E view works with this packing) but e4m3 quantizes
    E or V at 3.6% RMS > the 2e-2 budget; all hi/lo compensation
    schemes land back at fp16 cost.  fp32r = fp16 cost at >=256 cols.
  - ~40 perturbations each measure WORSE than defaults: OFF_BIG 6/7,
    HSPLIT, PIPE_DEPTH 2/4, TRI_DVE/TRI_MIX, OFF_SHIFT scan (8 of 15),
    DIAGFIRST, CW 512 (158us: po serializes), MINJECT (PE +6.8us),
    ITEM_W 512 (+19us op overhead), hostdiv/actcp/dvecp epilogues,
    KT_POOL prefetch routing, E/EO_BUFS, V0_POOL, Bresenham and custom
    K_OFF_LIST placements (incl. per-bh alternation).
  - WHY OFF_BIG=5 is the constrained optimum (the spacing rule): a
    DVE-exp'd item occupies the DVE ~1.19us but the PE produces one ST
    item per ~0.643us through a 3-deep ST rotation, so DVE items must
    sit >=3 apart in the item sequence or the exp(n-3)->QK(n) ST-release
    chain stalls the PE and then the ACT (measured: OFF=6 forces two
    spacing-2 pairs by pigeonhole -> +1.8us periodic stall per bh).
    Max items at spacing >=3 on a 17-cycle = 5 -> ACT >= 12 items/head
    = 100.9us busy.  ST rotation cannot deepen: PSUM = 8 banks = 3x2 ST
    + 2x1 po, and accumulators cannot share a bank (ZERO_REGION_SIZE =
    2048 = full bank, so a start=True matmul would wipe a cohabitant).
    Makespan 113.4 = 100.9 + startup 3.5 (preamble 0.7 + 2-queue DMA
    chain ~2.7, both near fixed-cost floors) + residual ACT gaps 4.5
    + drain tail 4.2 (~3.2 floor).  PE first-2-matmul pstate penalty is
    only ~108ns (pe_busy_start is set by the preamble Drain).
"""
import sys

if '/opt/trn_rl_repo' not in sys.path:
    sys.path.insert(0, '/opt/trn_rl_repo')

import os

import numpy as np

import concourse.bacc as bacc
import concourse.bass as bass
import concourse.mybir as mybir
import concourse.tile as tile
from concourse import masks

B, H, S, D = 4, 16, 2048, 64
N_CORES = 8
BH_PER_CORE = (B * H) // N_CORES  # 8
SCALE = float(D) ** -0.5
P = 128
NT = S // P  # 16 key/query tiles per (b,h)
CW = int(os.environ.get("K_CW", "128"))  # i-chunk width (CW/128 PSUM acc banks)
NC_CHUNK = S // CW
F16 = mybir.dt.float16
F32 = mybir.dt.float32

# ST/exp item width (PSUM score-tile columns)
ITEM_W = int(os.environ.get("K_ITEM_W", "1024"))
E_BUFS = int(os.environ.get("K_E_BUFS", "10"))
PIPE_DEPTH = int(os.environ.get("K_PIPE_DEPTH", "3"))
# number of exp items per (b,h) offloaded to the DVE via the Schraudolph
# fp16 bit-trick (int16(x*A+B) bitcast to fp16), balancing ACT vs DVE
OFF_BIG = int(os.environ.get("K_OFFBIG", "5"))
# exp items per (b,h) on the Pool engine: MUST stay 0 — the BIR verifier
# forbids GPSIMD (Pool) access to PSUM, and the scores live in PSUM
POOL_BIG = int(os.environ.get("K_POOLBIG", "0"))
# chunk-epilogue style: "actcp" = ACT copies numerator+denominator
# PSUM->SBUF (fast po-bank release), then a gpsimd SBUF->SBUF divide on
# the otherwise-idle Pool engine; "dvecp" = same but the copy on DVE;
# "dve" = baseline DVE reciprocal + broadcast multiply
EPI_ENG = os.environ.get("K_EPI_ENG", "dve")
# SPLIT mode: every item's exp is column-split between ACT ([0, c)) and
# DVE Schraudolph ([c, ew)); c = round(ew * SPLIT_F) to a 128 multiple.
SPLIT = int(os.environ.get("K_SPLIT", "0"))
SPLIT_F = float(os.environ.get("K_SPLIT_F", "0.60"))
SCH_A16 = float(2**10 / np.log(2)) * SCALE
SCH_B16 = float(15 * 2**10) - 60.0
PREP_AT = int(os.environ.get("K_PREP_AT", "-1"))  # -1: 3/4 through the items
DVE_LAG = int(os.environ.get("K_DVE_LAG", "2"))
POOL_LAG = int(os.environ.get("K_POOL_LAG", "3"))
# PSUM output-accumulator bufs (1 bank per CW//P chunks); ITEM_W=512 ->
# 1-bank ST tiles -> 6 ST bufs + 2 po bufs fill the 8 banks
_PO_BUFS_DEF = 2 if CW == P else 1


def _build_items():
    """Pack causal score units into [128, ITEM_W] ST tiles.

    unit = (ci, jb, u0, u1, eoff): score block for keys [jb*P, (jb+1)*P)
    x queries [u0, u1), placed at column eoff of its ST tile.  Units are
    packed greedily; a unit never crosses a 512-column PSUM bank
    boundary inside the tile.  Returns a list of items, one per ST tile:
    (units, ew) with ew = total exp width.
    """
    items = []
    cur, off = [], 0
    close_chunks = CW > P  # single-buffered accumulator: drain in order
    for ci in range(NC_CHUNK):
        c0, c1 = ci * CW, (ci + 1) * CW
        for jb in range(c1 // P):
            u0 = max(c0, jb * P)
            w = c1 - u0
            o = off
            if o // 512 != (o + w - 1) // 512:
                o = (o // 512 + 1) * 512
            if o + w > ITEM_W:
                items.append((cur, off))
                cur, o = [], 0
            cur.append((ci, jb, u0, c1, o))
            off = o + w
        if close_chunks and cur:
            items.append((cur, off))
            cur, off = [], 0
    if cur:
        items.append((cur, off))
    return items


def build_nc():
    nc = bacc.Bacc()
    qt_ext = nc.declare_dram_parameter("qt", [BH_PER_CORE, D, S], F16, isOutput=False)
    kt_ext = nc.declare_dram_parameter("kt", [BH_PER_CORE, D, S], F16, isOutput=False)
    v_ext = nc.declare_dram_parameter("v", [BH_PER_CORE, S, D + 1], F16, isOutput=False)
    # hostdiv epilogue emits [numerator | denominator] (D+1 cols); the final
    # divide happens on the host, like the layout prep already does
    OUT_W = D + 1 if EPI_ENG == "hostdiv" else D
    out_ext = nc.declare_dram_parameter("out", [BH_PER_CORE, S, OUT_W], F16, isOutput=True)

    MINJECT = int(os.environ.get("K_MINJECT", "0"))
    mb_ext = (nc.declare_dram_parameter("mb", [P, P], F16, isOutput=False)
              if MINJECT else None)

    items = _build_items()
    n_items = len(items)
    # items eligible for DVE offload: no diagonal unit (those need the
    # gpsimd trimask which works for either path, but keeping diagonal
    # items on ACT keeps the DVE batches uniform), decent width
    min_ew = min(512, ITEM_W // 2 + 1)
    tail_act = int(os.environ.get("K_TAIL_ACT", "0"))
    hi = n_items - tail_act
    if CW > P:
        cands = [i for i, (us, ew) in enumerate(items)
                 if ew >= min_ew and i < hi
                 and all(u0 != jb * P for (_, jb, u0, _, _) in us)]
    else:
        # trimask (applied post-exp) handles diagonal units on either path
        cands = [i for i, (us, ew) in enumerate(items)
                 if 0 < i < hi and ew >= min_ew]
    off_set_bh = None  # per-(b,h) override, set by K_OFF_LIST
    if int(os.environ.get("K_ASSIGN_BRES", "0")):
        # weighted round-robin (Bresenham) over the item sequence: every
        # engine's items are near-evenly SPACED, so no local engine overload
        # (two adjacent ACT items cost 2x621ns vs ~640ns of PE time)
        cset = set(cands)
        off_l, pool_l = [], []
        na = n_items - OFF_BIG - POOL_BIG
        err = {"a": 0.0, "d": 0.0, "p": 0.0}
        share = {"a": na / n_items, "d": OFF_BIG / n_items,
                 "p": POOL_BIG / n_items}
        for i in range(n_items):
            for e in err:
                err[e] += share[e]
            order = sorted(err, key=lambda e: -err[e])
            if i not in cset:
                pick = "a"
            else:
                for pick in order:
                    if pick == "a":
                        break
                    if pick == "d" and len(off_l) < OFF_BIG:
                        break
                    if pick == "p" and len(pool_l) < POOL_BIG:
                        break
            err[pick] -= 1.0
            if pick == "d":
                off_l.append(i)
            elif pick == "p":
                pool_l.append(i)
        off_set, pool_set = set(off_l), set(pool_l)
    elif os.environ.get("K_OFF_LIST"):
        # explicit DVE item indices: DVE items must be >=3 apart in the
        # item sequence (ST rotation = 3 tiles; two adjacent-ish DVE exps
        # at 1.19us each outrun PE's 0.643us/item and stall the pipeline);
        # a forced spacing-2 pair hides best across the bh boundary.
        # ';'-separated lists alternate per (b,h) on the core.
        _lists = [set(int(x) for x in part.split(","))
                  for part in os.environ["K_OFF_LIST"].split(";")]
        off_set = _lists[0]
        off_set_bh = lambda bh: _lists[bh % len(_lists)]  # noqa: E731
        pool_set = set()
    else:
        _soff = int(os.environ.get("K_OFF_SHIFT", "1"))
        off_set = set(
            cands[(round(i * len(cands) / OFF_BIG) + _soff) % len(cands)]
            for i in range(OFF_BIG)) if OFF_BIG else set()
        # Pool-exp items drawn from the remaining candidates
        restp = [i for i in cands if i not in off_set]
        _poff = int(os.environ.get("K_POOL_SHIFT", "0"))
        pool_set = set(
            restp[(round(i * len(restp) / POOL_BIG) + _poff) % len(restp)]
            for i in range(POOL_BIG)) if POOL_BIG else set()
    # hybrid ACT-head/DVE-tail items drawn from the remaining ACT items.
    # K_HS_LIST places them explicitly: a hybrid's DVE-tail op (~659ns at
    # a 0.5 split) fits a spacing-2 window that a full DVE item (1.19us)
    # cannot, so one hybrid can sit at the spacing boundary and relieve
    # the ACT beyond the 5-full-DVE-items/head cap
    HSPLIT = int(os.environ.get("K_HSPLIT", "0"))
    HS_F = float(os.environ.get("K_HS_F", "0.6"))
    if os.environ.get("K_HS_LIST"):
        hsplit_set = set(int(x) for x in os.environ["K_HS_LIST"].split(","))
    else:
        rest = [i for i in cands if i not in off_set and i not in pool_set]
        hsplit_set = set(
            rest[(round(i * len(rest) / HSPLIT)) % len(rest)]
            for i in range(HSPLIT)) if HSPLIT else set()

    with tile.TileContext(nc) as tc:
        with (
            tc.tile_pool(name="const", bufs=1) as const_pool,
            tc.tile_pool(name="io", bufs=int(os.environ.get("K_IO_BUFS", "2"))) as io_pool,
            tc.tile_pool(name="ep", bufs=E_BUFS) as e_pool,
            tc.tile_pool(name="eo", bufs=int(os.environ.get("K_EO_BUFS", "3"))) as eo_pool,
            tc.tile_pool(name="ps", bufs=int(os.environ.get(
                "K_ST_BUFS",
                str((8 - CW // P * _PO_BUFS_DEF) // (ITEM_W * 4 // 2048)))),
                space="PSUM") as ps_pool,
            tc.tile_pool(name="po", bufs=int(os.environ.get(
                "K_PO_BUFS", str(_PO_BUFS_DEF))), space="PSUM") as po_pool,
        ):

            def load(bh):
                """Returns (q_ap, k_ap, v_ap) accessor closures.

                For bh 0 the first 4 i/j-tiles live in separate head tiles
                with their own small DMAs, so the first items' scores only
                depend on ~1/4 of the load (whole-tile dependency
                granularity would otherwise delay the PE by ~3us).
                """
                v_view = v_ext[bh].rearrange("(t p) c -> p t c", p=P)
                if bh == 0 and int(os.environ.get('K_SPLITLOAD', '1')):
                    q0 = io_pool.tile([D, 512], F16, tag="qt0", name="qt0")
                    k0 = io_pool.tile([D, 512], F16, tag="kt0", name="kt0")
                    v0 = io_pool.tile([P, 4, D + 1], F16, tag="vp0",
                                      name="vp0")
                    q1 = io_pool.tile([D, S - 512], F16, tag="qt", name="qt")
                    k1 = io_pool.tile([D, S - 512], F16, tag="kt", name="kt")
                    v1 = io_pool.tile([P, NT - 4, D + 1], F16, tag="vp",
                                      name="vp")
                    nc.sync.dma_start(out=q0, in_=qt_ext[bh][:, :512])
                    if int(os.environ.get('K_K0_POOL', '1')):
                        nc.gpsimd.dma_start(out=k0, in_=kt_ext[bh][:, :512])
                    else:
                        nc.sync.dma_start(out=k0, in_=kt_ext[bh][:, :512])
                    if int(os.environ.get('K_V0_POOL', '0')):
                        nc.gpsimd.dma_start(out=v0, in_=v_view[:, :4])
                    else:
                        nc.sync.dma_start(out=v0, in_=v_view[:, :4])
                    nc.sync.dma_start(out=q1, in_=qt_ext[bh][:, 512:])
                    if int(os.environ.get('K_K1_POOL', '1')):
                        nc.gpsimd.dma_start(out=k1, in_=kt_ext[bh][:, 512:])
                    else:
                        nc.sync.dma_start(out=k1, in_=kt_ext[bh][:, 512:])
                    nc.sync.dma_start(out=v1, in_=v_view[:, 4:])
                    return (
                        lambda u0, u1: q0[:, u0:u1] if u1 <= 512
                        else q1[:, u0 - 512:u1 - 512],
                        lambda jb: k0[:, jb * P:(jb + 1) * P] if jb < 4
                        else k1[:, (jb - 4) * P:(jb - 3) * P],
                        lambda jb: v0[:, jb, :] if jb < 4
                        else v1[:, jb - 4, :],
                    )
                qt = io_pool.tile([D, S], F16, tag="qt", name="qt")
                kt = io_pool.tile([D, S], F16, tag="kt", name="kt")
                vp = io_pool.tile([P, NT, D + 1], F16, tag="vp", name="vp")
                nc.sync.dma_start(out=qt, in_=qt_ext[bh])
                # K_KT_POOL routes the prefetch loads via the gpsimd SWDGE
                # queue, bypassing the SP queue where they sit behind the
                # current head's output-DMA dispatches (565ns SEQ each)
                ktq = int(os.environ.get('K_KT_POOL', '0'))
                (nc.gpsimd if ktq >= 1 else nc.sync).dma_start(
                    out=kt, in_=kt_ext[bh])
                (nc.gpsimd if ktq >= 2 else nc.sync).dma_start(
                    out=vp, in_=v_view)
                return (lambda u0, u1: qt[:, u0:u1],
                        lambda jb: kt[:, jb * P:(jb + 1) * P],
                        lambda jb: vp[:, jb, :])

            loads = {0: load(0)}
            # consts built after the first loads are queued: DMA dispatch
            # overlaps the ACT table load / mask generation
            warm = const_pool.tile([P, 1], F32)
            nc.vector.memset(warm, 0.0)
            nc.scalar.activation(out=warm, in_=warm,
                                 func=mybir.ActivationFunctionType.Exp)
            # keep-mask for the diagonal score tile: 1 where j_local <= i_local
            trimask = const_pool.tile([P, P], F16)
            masks.make_upper_triangular(nc, trimask, val=1.0, diag=True)
            if MINJECT:
                # mask-bias (0 keep / -4000 masked) and identity for the
                # PE mask-injection matmul on ACT-path diagonal units
                mbias = const_pool.tile([P, P], F16)
                nc.sync.dma_start(out=mbias, in_=mb_ext[:])
                ident = const_pool.tile([P, P], F16)
                masks.make_identity(nc, ident)
            for bh in range(BH_PER_CORE):
                if off_set_bh is not None:
                    off_set = off_set_bh(bh)
                q_ap, k_ap, v_ap = loads.pop(bh)

                ot = None        # current chunk accumulator PSUM tile
                ot_ci = -1
                stage = []       # [(units, e_sb), ...] pipelined
                osb = {"t": None}  # batched output staging across OGRP chunks
                OGRP = max(1, 512 // CW)  # chunks per output DMA

                def epilogue(ci, o):
                    """Drain a finished chunk into a staging tile; DMA once
                    per OGRP chunks."""
                    ntile = CW // P
                    g = ci % OGRP
                    if g == 0:
                        osb["t"] = eo_pool.tile([P, OGRP * ntile, OUT_W], F16,
                                                tag="o_sb", name="o_sb")
                        if EPI_ENG in ("actcp", "dvecp"):
                            osb["n"] = eo_pool.tile(
                                [P, OGRP * ntile, D + 1], F32, tag="o65",
                                name="o65")
                    o_sb = osb["t"]
                    if EPI_ENG == "hostdiv":
                        # single fp16 copy of numerator|denominator: the po
                        # bank is released after one 65-col op; the divide
                        # happens on the host during unshard
                        nc.vector.tensor_scalar_add(
                            o_sb[:, g * ntile:(g + 1) * ntile, :],
                            o[:, :, :D + 1], 0.0)
                    elif EPI_ENG in ("actcp", "dvecp"):
                        # copy numerator|denominator PSUM->SBUF (releases the
                        # po bank quickly), then divide SBUF->SBUF on the
                        # otherwise-idle Pool engine (gpsimd can't read PSUM)
                        o65 = osb["n"]
                        dst = o65[:, g * ntile:(g + 1) * ntile, :]
                        if EPI_ENG == "actcp":
                            nc.scalar.activation(
                                out=dst, in_=o[:, :, :D + 1],
                                func=mybir.ActivationFunctionType.Copy)
                        else:
                            nc.vector.tensor_scalar_add(dst, o[:, :, :D + 1],
                                                        0.0)
                        den = o65[:, g * ntile:(g + 1) * ntile, D]
                        den_b = bass.AP(tensor=den.tensor, offset=den.offset,
                                        ap=[den.ap[0], den.ap[1], [0, D]])
                        nc.gpsimd.tensor_tensor(
                            out=o_sb[:, g * ntile:(g + 1) * ntile, :],
                            in0=o65[:, g * ntile:(g + 1) * ntile, :D],
                            in1=den_b, op=mybir.AluOpType.divide)
                    else:
                        # rcp to SBUF then a broadcast multiply: the multiply
                        # may read only one PSUM input (DVE ISA)
                        rcp = eo_pool.tile([P, ntile], F32, tag="rcp",
                                           name="rcp")
                        nc.vector.reciprocal(out=rcp, in_=o[:, :, D])
                        rcp_b = bass.AP(tensor=rcp.tensor, offset=rcp.offset,
                                        ap=[rcp.ap[0], rcp.ap[1], [0, D]])
                        nc.vector.tensor_tensor(
                            out=o_sb[:, g * ntile:(g + 1) * ntile, :],
                            in0=o[:, :, :D], in1=rcp_b,
                            op=mybir.AluOpType.mult)
                    if g == OGRP - 1:
                        c0 = (ci - OGRP + 1) * CW
                        nc.sync.dma_start(
                            out=out_ext[bh, c0:c0 + OGRP * CW].rearrange(
                                "(t p) d -> p t d", p=P),
                            in_=o_sb)

                pv_seen = {}  # i-tile -> number of PV accumulations issued

                def flush_pv(units, e_sb):
                    nonlocal ot, ot_ci
                    # within each chunk, issue the diagonal unit's PV first:
                    # the masked E (gpsimd trimask) then sits early in the
                    # accumulation group instead of gating the group close
                    if int(os.environ.get('K_DIAGFIRST', '0')):
                        units = sorted(
                            units, key=lambda u: (u[0], u[2] != u[1] * P, u[1]))
                    for (ci, jb, u0, u1, eoff) in units:
                        if ci != ot_ci:
                            if ot is not None:
                                epilogue(ot_ci, ot)
                            ot = po_pool.tile([P, CW // P, 512], F32,
                                              tag="ot", name="ot")
                            ot_ci = ci
                        for t in range(u0 // P, u1 // P):
                            n = pv_seen.get(t, 0)
                            pv_seen[t] = n + 1
                            nc.tensor.matmul(
                                ot[:, t - ci * (CW // P), 0:D + 1],
                                e_sb[:, eoff + t * P - u0:eoff + t * P - u0 + P],
                                v_ap(jb),
                                start=(n == 0), stop=(n == t))

                prep_at = (PREP_AT if PREP_AT >= 0
                           else round(n_items * 3 / 4))
                for pidx, (units, ew) in enumerate(items):
                    act_item = (pidx not in off_set and pidx not in hsplit_set
                                and pidx not in pool_set)
                    st = ps_pool.tile([P, ITEM_W], F32, tag="st")
                    for (ci, jb, u0, u1, eoff) in units:
                        diag = u0 == jb * P
                        inject = MINJECT and act_item and diag
                        nc.tensor.matmul(
                            st[:, eoff:eoff + (u1 - u0)],
                            k_ap(jb), q_ap(u0, u1),
                            start=True, stop=not inject)
                        if inject:
                            # accumulate ident.T @ mbias = mbias into the
                            # diagonal block: exp underflows to exact 0 on
                            # the masked side, no post-exp trimask needed
                            nc.tensor.matmul(
                                st[:, eoff:eoff + P], ident, mbias,
                                start=False, stop=True)
                    # flush BEFORE emitting this item's exp: any chunk
                    # epilogue triggered by the flush then precedes later
                    # exps in the in-order queues, releasing the PSUM
                    # accumulator banks as early as possible.  DVE/Pool exp
                    # items get extra items of lead (their exp has higher
                    # latency than ACT's).
                    if stage and len(stage) >= PIPE_DEPTH + stage[0][0]:
                        flush_pv(*stage.pop(0)[1:])
                    if SPLIT:
                        ei = e_pool.tile([P, ITEM_W], mybir.dt.int16, tag="e",
                                         name="ei")
                        e_sb = ei.bitcast(F16)
                        c = int(round(ew * SPLIT_F / P)) * P
                        c = max(0, min(ew, c))
                        if c > 0:
                            nc.scalar.activation(
                                out=e_sb[:, :c], in_=st[:, :c],
                                func=mybir.ActivationFunctionType.Exp,
                                scale=SCALE)
                        if c < ew:
                            nc.vector.tensor_scalar(
                                out=ei[:, c:ew], in0=st[:, c:ew],
                                scalar1=SCH_A16, scalar2=SCH_B16,
                                op0=mybir.AluOpType.mult,
                                op1=mybir.AluOpType.add)
                    elif pidx in hsplit_set:
                        # hybrid: ACT takes the head, a short DVE
                        # Schraudolph op takes the tail columns
                        ei = e_pool.tile([P, ITEM_W], mybir.dt.int16, tag="e",
                                         name="ei")
                        e_sb = ei.bitcast(F16)
                        c = int(round(ew * HS_F / P)) * P
                        c = max(P, min(ew, c))
                        nc.scalar.activation(
                            out=e_sb[:, :c], in_=st[:, :c],
                            func=mybir.ActivationFunctionType.Exp,
                            scale=SCALE)
                        if c < ew:
                            nc.vector.tensor_scalar(
                                out=ei[:, c:ew], in0=st[:, c:ew],
                                scalar1=SCH_A16, scalar2=SCH_B16,
                                op0=mybir.AluOpType.mult,
                                op1=mybir.AluOpType.add)
                    elif pidx in off_set or pidx in pool_set:
                        ei = e_pool.tile([P, ITEM_W], mybir.dt.int16, tag="e",
                                         name="ei")
                        exp_eng = (nc.gpsimd if pidx in pool_set
                                   else nc.vector)
                        exp_eng.tensor_scalar(
                            out=ei[:, :ew], in0=st[:, :ew],
                            scalar1=SCH_A16, scalar2=SCH_B16,
                            op0=mybir.AluOpType.mult, op1=mybir.AluOpType.add)
                        e_sb = ei.bitcast(F16)
                    else:
                        e_sb = e_pool.tile([P, ITEM_W], F16, tag="e")
                        nc.scalar.activation(
                            out=e_sb[:, :ew], in_=st[:, :ew],
                            func=mybir.ActivationFunctionType.Exp, scale=SCALE)
                    for (ci, jb, u0, u1, eoff) in units:
                        if u0 == jb * P and not (MINJECT and act_item):
                            # diagonal tile: post-exp causal keep-mask.
                            # TRI_MIX: DVE-exp'd items mask on DVE (adjacent
                            # in-queue after their exp, no Pool round-trip in
                            # the PV-close path); ACT items keep Pool
                            if int(os.environ.get("K_TRI_DVE", "0")):
                                eng = nc.vector
                            elif (int(os.environ.get("K_TRI_MIX", "0"))
                                  and not act_item):
                                eng = nc.vector
                            else:
                                eng = nc.gpsimd
                            eng.tensor_mul(
                                e_sb[:, eoff:eoff + P],
                                e_sb[:, eoff:eoff + P], trimask)
                    lag = (POOL_LAG if pidx in pool_set
                           else DVE_LAG if pidx in off_set else 0)
                    stage.append((lag, units, e_sb))
                    if pidx == prep_at and bh + 1 < BH_PER_CORE:
                        loads[bh + 1] = load(bh + 1)
                while stage:
                    flush_pv(*stage.pop(0)[1:])
                epilogue(ot_ci, ot)

    nc.compile()
    return nc


_CACHE = {}


def _get_runner():
    """Build + compile once; return a cached jitted 8-core runner."""
    if "runner" in _CACHE:
        return _CACHE["runner"]

    import jax
    from jax.sharding import Mesh, PartitionSpec
    from jax.experimental.shard_map import shard_map
    from concourse import bass2jax
    from concourse.bass2jax import _bass_exec_p, partition_id_tensor
    import concourse.mybir as _mybir

    nc = build_nc()
    bass2jax.install_neuronx_cc_hook()

    partition_name = nc.partition_id_tensor.name if nc.partition_id_tensor else None
    in_names, out_names, out_avals = [], [], []
    for alloc in nc.m.functions[0].allocations:
        if not isinstance(alloc, _mybir.MemoryLocationSet):
            continue
        name = alloc.memorylocations[0].name
        if alloc.kind == "ExternalInput":
            if name != partition_name:
                in_names.append(name)
        elif alloc.kind == "ExternalOutput":
            shape = tuple(alloc.tensor_shape)
            dtype = _mybir.dt.np(alloc.dtype)
            out_names.append(name)
            out_avals.append(jax.core.ShapedArray(shape, dtype))
    n_params = len(in_names)
    all_names = list(in_names) + list(out_names)
    if partition_name is not None:
        all_names.append(partition_name)

    def _body(*args):
        operands = list(args)
        if partition_name is not None:
            operands.append(partition_id_tensor())
        outs = _bass_exec_p.bind(
            *operands,
            out_avals=tuple(out_avals),
            in_names=tuple(all_names),
            out_names=tuple(out_names),
            lowering_input_output_aliases=(),
            sim_require_finite=True,
            sim_require_nnan=True,
            nc=nc,
        )
        return tuple(outs)

    devices = jax.devices()[:N_CORES]
    mesh = Mesh(np.asarray(devices), ("core",))
    n_outs = len(out_names)
    in_specs = (PartitionSpec("core"),) * (n_params + n_outs)
    out_specs = (PartitionSpec("core"),) * n_outs
    sharded = jax.jit(shard_map(
        _body, mesh=mesh, in_specs=in_specs, out_specs=out_specs,
        check_rep=False))

    runner = {
        "fn": sharded,
        "in_names": in_names,
        "out_names": out_names,
        "out_avals": out_avals,
        "mesh": mesh,
    }
    _CACHE["runner"] = runner
    return runner


def _prep(q, k, v):
    """Host layout prep: [B,H,S,D] f32 -> per-core concatenated fp16 DRAM
    layouts (qT/kT d-major, v with fused ones column)."""
    qf = q.reshape(B * H, S, D).astype(np.float16)
    kf = k.reshape(B * H, S, D).astype(np.float16)
    vf = v.reshape(B * H, S, D).astype(np.float16)
    qt = np.ascontiguousarray(np.swapaxes(qf, 1, 2))  # [BH, D, S]
    kt = np.ascontiguousarray(np.swapaxes(kf, 1, 2))
    v65 = np.concatenate(
        [vf, np.ones((B * H, S, 1), dtype=np.float16)], axis=-1)
    jj = np.arange(P)[:, None]
    ii = np.arange(P)[None, :]
    mb = np.where(jj > ii, np.float16(-4000.0), np.float16(0.0))
    mb = np.broadcast_to(mb.astype(np.float16), (P, P))
    return {"qt": qt, "kt": kt, "v": np.ascontiguousarray(v65),
            "mb": np.ascontiguousarray(mb)}


def kernel(q, k, v):
    q = np.asarray(q, dtype=np.float32)
    k = np.asarray(k, dtype=np.float32)
    v = np.asarray(v, dtype=np.float32)
    r = _get_runner()
    ins = _prep(q, k, v)
    concat_in = [ins[name] for name in r["in_names"]]
    zeros = [np.zeros((N_CORES * av.shape[0],) + av.shape[1:], av.dtype)
             for av in r["out_avals"]]
    outs = r["fn"](*concat_in, *zeros)
    out = np.asarray(outs[r["out_names"].index("out")])
    if out.shape[-1] == D + 1:
        # hostdiv epilogue: device emitted [numerator | denominator]
        out = out.astype(np.float32)
        out = out[..., :D] / out[..., D:D + 1]
        return out.reshape(B, H, S, D)
    return out.astype(np.float32).reshape(B, H, S, D)

